# revision 2
# baseline (speedup 1.0000x reference)
"""Trainium2 Bass kernel for nn_AttentionBlock (GroupNorm -> MHA -> proj + residual).

Contract: kernel(**inputs) takes the FULL unsharded inputs (as produced by
setup_inputs) and returns the FULL output [8, 512, 32, 32] float32.

Sharding: pure data-parallel over batch B=8 across the 8 NeuronCores; each core
processes one batch element end-to-end (no collectives needed).

v3, informed by HW microbenchmarks: a [*,512]-out matmul costs ~262 ns
regardless of dtype/DoubleRow (fp8 DR doubles contraction per instruction,
not row rate), and 64-partition matmuls are ~1.8x SLOWER (468 ns). So:
  - S (QK^T): plain bf16 at FULL 128 partitions -- k is stored zero-padded
    per head (kz: head's 64 channels live on their own partitions, the other
    64 partitions zeroed) so q can stay dense; the zero partitions kill the
    other head's contribution. This beats both the baseline's 64-partition
    matmuls (468 ns) and fp8-DR zero-padded (also 64-partition).
  - AV and proj: fp8e4 DoubleRow -- 2x contraction per instruction halves
    the instruction count at equal per-instruction cost.
  - exp runs on ScalarE with bias -4ln2 so E fits in fp8e4 (max 448),
    writing fp8 directly; the 2^-4 scale cancels between the AV numerator
    and the ones-column denominators (ones FIRST: partitions 0:64, where the
    custom-DVE reciprocal reads correctly -- it misreads offset sources).
  - qkv matmuls bf16 (fp8 h/w pushes rel-err too close to the 2e-2 gate).

Per-core layout (B=1, C=512, N=1024, heads=8, head_dim=64):
  - GroupNorm(32 groups): bn_stats/bn_aggr + tiny PE combine/broadcast
    matmuls; h in bf16.
  - S^T units [128 m, 512 n] per (head, n-half, m-tile): bf16, kz-padded.
  - exp: ScalarE -> fp8 E ring [128, 2(phase parity), 16, 512].
  - AV: fp8 DR over m-tile pairs, ones-columns give softmax denominators.
  - epilogue: DVE reciprocal on PSUM denominators + mult -> O_sb fp8.
  - proj: fp8 DR + scalar_tensor_tensor residual add (x + pb + psum).
"""

import numpy as np
import ml_dtypes

import concourse.bass as bass
import concourse.tile as tile
from concourse import bacc, mybir
from concourse.bass_utils import run_bass_kernel_spmd

FP32 = mybir.dt.float32
BF16 = mybir.dt.bfloat16
FP8 = mybir.dt.float8e4
AF = mybir.ActivationFunctionType
OP = mybir.AluOpType
PM = mybir.MatmulPerfMode

P = 128      # SBUF partitions
C = 512      # channels
NT = 1024    # spatial tokens (32*32)
CT = C // P  # channel tiles = 4
MT = NT // P # m (key) tiles = 8
NH = 8       # heads
HD = 64      # head dim
NCORES = 8
GSZ = 16     # channels per group (512/32)

LN2X4 = float(4.0 * np.log(2.0))


def _emit(tc: "tile.TileContext", io: dict):
    nc = tc.nc
    x, wq, wk, wv, pw = io["x"], io["wq"], io["wk"], io["wv"], io["pw"]
    bq, bk, pb = io["bq"], io["bk"], io["pb"]
    gg, gb = io["gg"], io["gb"]
    amat, imat = io["amat"], io["imat"]
    ones8 = io["ones8"]
    out = io["out"]

    import contextlib
    ctx = contextlib.ExitStack()
    with ctx:
        pers = ctx.enter_context(tc.tile_pool(name="pers", bufs=1))
        sm = ctx.enter_context(tc.tile_pool(name="small", bufs=1))

        # ---------------- input DMAs ----------------
        # x is the critical path: first on both the sync and gpsimd queues.
        # Small constants + zero/ones fills ride the scalar queue, which is
        # idle until the first attention exp (~15us in).
        x_r = x.rearrange("(r p) n -> p r n", p=P)
        x_sb = pers.tile([P, CT, NT], FP32, tag="x")
        nc.sync.dma_start(x_sb[:, 0, :], x_r[:, 0, :])
        nc.gpsimd.dma_start(x_sb[:, 1, :], x_r[:, 1, :])
        nc.sync.dma_start(x_sb[:, 2, :], x_r[:, 2, :])
        nc.gpsimd.dma_start(x_sb[:, 3, :], x_r[:, 3, :])
        amat_sb = pers.tile([P, NH], FP32, tag="amat")
        nc.scalar.dma_start(amat_sb, amat)
        imat_sb = pers.tile([NH, P], FP32, tag="imat")
        nc.scalar.dma_start(imat_sb, imat)
        gg_sb = pers.tile([P, CT], FP32, tag="gg")
        nc.scalar.dma_start(gg_sb, gg.rearrange("(r p) -> p r", p=P))
        gb_sb = pers.tile([P, CT], FP32, tag="gb")
        nc.scalar.dma_start(gb_sb, gb.rearrange("(r p) -> p r", p=P))
        bq_sb = pers.tile([P, CT], FP32, tag="bq")
        nc.scalar.dma_start(bq_sb, bq.rearrange("(r p) -> p r", p=P))
        bk_sb = pers.tile([P, CT], FP32, tag="bk")
        nc.scalar.dma_start(bk_sb, bk.rearrange("(r p) -> p r", p=P))
        pb_sb = pers.tile([P, CT], FP32, tag="pb")
        nc.scalar.dma_start(pb_sb, pb.rearrange("(r p) -> p r", p=P))

        # q dense bf16; k zero-partition-padded per head (kz): slot (pr, hi)
        # holds head 2pr+hi's channels on partitions 64*hi..64*hi+64, zeros
        # on the other 64 partitions
        q_sb = pers.tile([P, CT, NT], BF16, tag="q")
        kz_sb = pers.tile([P, CT, 2, NT], BF16, tag="kz")
        # zero fills on the Pool engine: DMA bandwidth is reserved for x
        # and the weights during startup
        nc.gpsimd.memset(kz_sb[HD:128, :, 0, :], 0.0)
        nc.gpsimd.memset(kz_sb[0:HD, :, 1, :], 0.0)

        wq_sb = pers.tile([P, CT, C], BF16, tag="wq")
        nc.gpsimd.dma_start(wq_sb, wq.rearrange("(k p) o -> p k o", p=P))
        wk_sb = pers.tile([P, CT, C], BF16, tag="wk")
        nc.gpsimd.dma_start(wk_sb, wk.rearrange("(k p) o -> p k o", p=P))
        wv_sb = pers.tile([P, CT, C], BF16, tag="wv")
        nc.sync.dma_start(wv_sb, wv.rearrange("(k p) o -> p k o", p=P))
        pw_sb = pers.tile([P, CT, C], FP8, tag="pw")
        nc.sync.dma_start(pw_sb, pw.rearrange("(k p) o -> p k o", p=P))

        # preload the exp activation table while DMAs are in flight
        warm_sb = pers.tile([1, 1], FP32, tag="actwarm")
        nc.vector.memset(warm_sb, 0.0)
        nc.scalar.activation(warm_sb, warm_sb, AF.Exp)
        ebias_sb = pers.tile([P, 1], FP32, tag="ebias")
        nc.vector.memset(ebias_sb, -LN2X4)

        # v^T with interleaved ones columns: per head 128 cols = [ones(64)|v(64)]
        # (ones FIRST so the softmax denominators land on PSUM partitions 0:64,
        # where the custom-DVE reciprocal can read them -- it misreads
        # partition-offset sources). Whole-tile ones arrive by DMA after the
        # weights; first AV consumes it at ~25us which the AV-lag tolerates.
        vT_sb = pers.tile([P, MT, NH * 128], FP8, tag="vT")
        nc.gpsimd.dma_start(vT_sb, ones8.rearrange("p (t c) -> p t c", t=MT))

        h_sb = pers.tile([P, CT, NT], BF16, tag="h")
        O_sb = pers.tile([P, CT, NT], FP8, tag="O")
        out_sb = pers.tile([P, CT, NT], FP32, tag="outsb")
        # E ring: [parity, 16 units (hi*8 + t), 512]
        ering = pers.tile([P, 2, 16, 512], FP8, tag="ering")

        def qk_into(ps, dst, w_sb, b_sb, r, half):
            # bf16 matmul chain -> bf16 eviction with bias add. q goes to
            # the dense q_sb; k splits into the two zero-padded kz slots
            # (head 2r -> partitions 0:64 of slot (r,0); head 2r+1 ->
            # partitions 64:128 of slot (r,1)).
            hs = 512 * half
            for kc in range(CT):
                nc.tensor.matmul(
                    ps, w_sb[:, kc, P * r:P * r + P],
                    h_sb[:, kc, hs:hs + 512],
                    start=(kc == 0), stop=(kc == CT - 1))
            if dst is q_sb:
                nc.vector.tensor_scalar(dst[:, r, hs:hs + 512],
                                        ps, b_sb[:, r:r + 1], None, OP.add)
            else:
                nc.vector.tensor_scalar(
                    dst[0:HD, r, 0, hs:hs + 512], ps[0:HD, :],
                    b_sb[0:HD, r:r + 1], None, OP.add)
                nc.vector.tensor_scalar(
                    dst[HD:128, r, 1, hs:hs + 512], ps[HD:128, :],
                    b_sb[HD:128, r:r + 1], None, OP.add)

        # ---------------- GroupNorm (h = bf16) ----------------
        # All combine-chain tensors use a k-major [*, 2, CT] layout so every
        # DVE op runs on a small CONTIGUOUS [*, CT] block (strided
        # tiny-inner-dim APs cost ~3us each on HW).
        with nc.named_scope("gn"), \
             tc.tile_pool(name="gnps", bufs=1, space="PSUM") as gnps, \
             tc.tile_pool(name="mrps", bufs=1, space="PSUM") as mrps:
            st2_all = sm.tile([P, 2, CT], FP32, tag="st2_all")
            mv_all = sm.tile([P, 2, CT], FP32, tag="mv_all")
            for r in range(CT):
                st = sm.tile([P, 2, 6], FP32, tag=f"bnstats{r}")
                nc.vector.bn_stats(st[:, 0, :], x_sb[:, r, 0:512])
                nc.vector.bn_stats(st[:, 1, :], x_sb[:, r, 512:1024])
                nc.vector.bn_aggr(mv_all[:, :, r], st)
            # st2 = [mean | mean^2 + var], all contiguous [P, CT] ops
            nc.vector.tensor_copy(st2_all[:, 0, :], mv_all[:, 0, :])
            nc.vector.tensor_tensor(st2_all[:, 1, :], mv_all[:, 0, :],
                                    mv_all[:, 0, :], OP.mult)
            nc.vector.tensor_tensor(st2_all[:, 1, :], st2_all[:, 1, :],
                                    mv_all[:, 1, :], OP.add)
            G_ps = gnps.tile([NH, 2, CT], FP32, tag="gps")
            nc.tensor.matmul(G_ps, amat_sb,
                             st2_all.rearrange("p k r -> p (k r)"),
                             start=True, stop=True)
            st_all = sm.tile([NH, 2, CT], FP32, tag="st_all")
            nc.vector.tensor_copy(st_all, G_ps)
            var_all = sm.tile([NH, CT], FP32, tag="var_all")
            nc.vector.tensor_tensor(var_all, st_all[:, 0, :],
                                    st_all[:, 0, :], OP.mult)
            nc.vector.tensor_tensor(var_all, st_all[:, 1, :],
                                    var_all, OP.subtract)
            nc.vector.tensor_scalar(var_all, var_all, 1e-5, None, OP.add)
            y = sm.tile([NH, CT], FP32, tag="rsqrt_y")
            nc.vector.reciprocal_approx_fast(y, var_all)
            t = sm.tile([NH, CT], FP32, tag="rsqrt_t")
            for it in range(2):
                nc.vector.tensor_tensor(t, y, y, OP.mult)
                nc.vector.tensor_tensor(t, t, var_all, OP.mult)
                nc.vector.tensor_scalar(t, t, -0.5, 1.5, OP.mult, OP.add)
                if it < 1:
                    nc.vector.tensor_tensor(y, y, t, OP.mult)
                else:
                    nc.vector.tensor_tensor(st_all[:, 1, :], y, t, OP.mult)
            MR_ps = mrps.tile([P, 2, CT], FP32, tag="mrps")
            nc.tensor.matmul(MR_ps, imat_sb,
                             st_all.rearrange("p k r -> p (k r)"),
                             start=True, stop=True)
            mr = sm.tile([P, 2, CT], FP32, tag="mr")
            nc.vector.tensor_copy(mr, MR_ps)
            a_all = sm.tile([P, CT], FP32, tag="gn_a")
            nc.vector.tensor_tensor(a_all, mr[:, 1, :], gg_sb, OP.mult)
            b_all = sm.tile([P, CT], FP32, tag="gn_b")
            nc.vector.tensor_tensor(b_all, mr[:, 0, :], a_all, OP.mult)
            nc.vector.tensor_tensor(b_all, gb_sb, b_all, OP.subtract)
            for r in range(CT):
                nc.vector.tensor_scalar(h_sb[:, r, :], x_sb[:, r, :],
                                        a_all[:, r:r + 1], b_all[:, r:r + 1],
                                        OP.mult, OP.add)



        # ------------- qkv + attention -------------
        # PSUM (8 banks): S tiles s3 [128,3,512] + s2 [128,2,512] (5) +
        # O [128,512] x1 (1) + bg accumulator [128,512] x2 (2). Double-
        # buffered bg lets qk/vt/proj tasks ping-pong banks so a task's
        # eviction (DVE) overlaps the next task's matmuls instead of
        # head-of-line blocking the PE queue.
        with nc.named_scope("qkv_attn"), \
             tc.tile_pool(name="bgps", bufs=2, space="PSUM") as bgps, \
             tc.tile_pool(name="spool", bufs=1, space="PSUM") as spool, \
             tc.tile_pool(name="opool", bufs=1, space="PSUM") as opool, \
             tc.tile_pool(name="rpool", bufs=2) as rpool:

            def qk_task(dst, w_sb, b_sb, r, half):
                ps = bgps.tile([P, 512], FP32, tag="bgps",
                               name=f"qk_{r}_{half}_{w_sb.name}")
                qk_into(ps, dst, w_sb, b_sb, r, half)

            # startup q/k for phase 0 on three PARALLEL banks: the first two
            # spool rotations plus the bg bank, so the three tasks don't
            # serialize through one bank's matmul -> evict -> WAR chain
            upA = spool.tile([P, 3, 512], FP32, tag="s3", name="up_q00")
            qk_into(upA[:, 0, :], q_sb, wq_sb, bq_sb, 0, 0)
            upB = spool.tile([P, 2, 512], FP32, tag="s2", name="up_k00")
            qk_into(upB[:, 0, :], kz_sb, wk_sb, bk_sb, 0, 0)
            upC = bgps.tile([P, 512], FP32, tag="bgps", name="up_k01")
            qk_into(upC, kz_sb, wk_sb, bk_sb, 0, 1)

            def vt_task(t):
                ps = bgps.tile([P, 512], FP32, tag="bgps", name=f"vt{t}")
                for kc in range(CT):
                    nc.tensor.matmul(ps, h_sb[:, kc, P * t:P * t + P],
                                     wv_sb[:, kc, :],
                                     start=(kc == 0), stop=(kc == CT - 1))
                nc.vector.tensor_copy(
                    vT_sb[:, t, :].rearrange("p (h c) -> p h c", c=128)[:, :, HD:128],
                    ps.rearrange("p (h c) -> p h c", c=HD))

            out_r = out.rearrange("(r p) n -> p r n", p=P)

            def proj_task(r, half):
                hs = 512 * half
                ps = bgps.tile([P, 512], FP32, tag="bgps",
                               name=f"pj_{r}_{half}")
                for jj in range(2):
                    nc.tensor.matmul(
                        ps, pw_sb[:, 2 * jj:2 * jj + 2, P * r:P * r + P],
                        O_sb[:, 2 * jj:2 * jj + 2, hs:hs + 512],
                        start=(jj == 0), stop=(jj == 1),
                        perf_mode=PM.DoubleRow)
                nc.vector.scalar_tensor_tensor(
                    out_sb[:, r, hs:hs + 512], ps, pb_sb[:, r:r + 1],
                    x_sb[:, r, hs:hs + 512], OP.add, OP.add)
                eng = nc.sync if (r + half) % 2 == 0 else nc.gpsimd
                eng.dma_start(out_r[:, r, hs:hs + 512], out_sb[:, r, hs:hs + 512])

            # phases in half-major order so proj(half=0) can run during the
            # half=1 phases. A phase (pr, half) consumes q(pr, half) and
            # k(pr, 0) AND k(pr, 1) (keys span all 1024 positions).
            phases = [(pr, 0) for pr in range(4)] + [(pr, 1) for pr in range(4)]

            def q_t(r, half):
                return (qk_task, (q_sb, wq_sb, bq_sb, r, half))

            def k_t(r, half):
                return (qk_task, (kz_sb, wk_sb, bk_sb, r, half))

            # 6 drip points per phase; k(pr,1) at point0 of phase (pr,0) is
            # ready before the t>=4 S-units of that phase. vt ordering is
            # enforced by the pending-AV queue, not the drip slots.
            drip = {
                0: [q_t(1, 0), k_t(1, 0), (vt_task, (0,)), (vt_task, (1,)),
                    (vt_task, (2,)), (vt_task, (3,))],
                1: [k_t(1, 1), q_t(2, 0), k_t(2, 0), (vt_task, (4,)),
                    (vt_task, (5,)), (vt_task, (6,)), (vt_task, (7,))],
                2: [k_t(2, 1), q_t(3, 0), k_t(3, 0)],
                3: [k_t(3, 1), q_t(0, 1), q_t(1, 1)],
                4: [q_t(2, 1), (proj_task, (0, 0)), (proj_task, (1, 0))],
                5: [q_t(3, 1), (proj_task, (2, 0)), (proj_task, (3, 0))],
                6: [],
                7: [],
            }

            def s_unit(S_t, slot, pr, half, hi, t):
                # S^T[m,n] for head 2pr+hi, m-tile t, n-half: bf16 at full
                # 128 partitions; kz's zero partitions cancel the other
                # head's q contribution
                nc.tensor.matmul(
                    S_t[:, slot, :],
                    kz_sb[:, pr, hi, P * t:P * t + P],
                    q_sb[:, pr, 512 * half:512 * half + 512],
                    start=True, stop=True)

            O_tiles = {}

            def av_pair(key, parity, pr, half, hi, j):
                if j == 0:
                    O_tiles[key] = opool.tile([P, 512], FP32, tag="oh",
                                              name=f"oh{key}")
                h = 2 * pr + hi
                nc.tensor.matmul(
                    O_tiles[key],
                    vT_sb[:, 2 * j:2 * j + 2, 128 * h:128 * h + 128],
                    ering[:, parity, 8 * hi + 2 * j:8 * hi + 2 * j + 2, :],
                    start=(j == 0), stop=(j == 3),
                    perf_mode=PM.DoubleRow)

            def epilogue(key, pr, half, hi):
                hs = 512 * half
                O_ps = O_tiles.pop(key)
                Rh = rpool.tile([HD, 512], FP32, tag="rh", name=f"rh{key}")
                nc.vector.reciprocal_approx_fast(Rh, O_ps[0:HD, :])
                nc.vector.tensor_tensor(
                    O_sb[HD * hi:HD * hi + HD, pr, hs:hs + 512],
                    O_ps[HD:128, :], Rh, OP.mult)

            from collections import deque
            pend = deque()   # ('av', key, parity, pr, half, hi, j) | ('epi',...)
            vt_emitted = set()

            def flush_ready():
                # flush AV pairs whose vT inputs have been emitted (FIFO);
                # epilogues follow their pair-3 AV in queue order
                while pend:
                    item = pend[0]
                    if item[0] == "av":
                        _, key, parity, pr, half, hi, j = item
                        if not (2 * j in vt_emitted and 2 * j + 1 in vt_emitted):
                            return
                        pend.popleft()
                        av_pair(key, parity, pr, half, hi, j)
                    else:
                        _, key, pr, half, hi = item
                        pend.popleft()
                        epilogue(key, pr, half, hi)

            for ph, (pr, half) in enumerate(phases):
                parity = ph % 2
                tasks = list(drip[ph])

                def drip_one():
                    if tasks:
                        fn, args = tasks.pop(0)
                        if fn is vt_task:
                            vt_emitted.add(args[0])
                        fn(*args)
                    flush_ready()

                for hi in range(2):
                    key = f"{ph}_{hi}"
                    for base, span, tg in ((0, 3, "s3"), (3, 2, "s2"),
                                           (5, 3, "s3")):
                        S_t = spool.tile([P, span, 512], FP32, tag=tg,
                                         name=f"st{ph}_{hi}_{base}")
                        for u in range(span):
                            s_unit(S_t, u, pr, half, hi, base + u)
                        nc.scalar.activation(
                            ering[:, parity, 8 * hi + base:8 * hi + base + span, :],
                            S_t[:, 0:span, :], AF.Exp, bias=ebias_sb)
                        # queue AV pairs fully covered by exps so far
                        if base == 0:
                            pend.append(("av", key, parity, pr, half, hi, 0))
                        elif base == 3:
                            pend.append(("av", key, parity, pr, half, hi, 1))
                        else:
                            pend.append(("av", key, parity, pr, half, hi, 2))
                            pend.append(("av", key, parity, pr, half, hi, 3))
                            pend.append(("epi", key, pr, half, hi))
                        drip_one()
                # leftover drip tasks at end of phase
                while tasks:
                    drip_one()
                flush_ready()
            assert not pend, "unflushed AV/epilogue work"

            # ---------------- proj tail: half=1 ----------------
            with nc.named_scope("proj"):
                for r in range(CT):
                    proj_task(r, 1)


_CACHE: dict = {}


def _build():
    if "nc" in _CACHE:
        return _CACHE["nc"]
    nc = bacc.Bacc("TRN2", target_bir_lowering=False, debug=False,
                   num_devices=NCORES)
    io = {
        "x": nc.dram_tensor("x", [C, NT], FP32, kind="ExternalInput").ap(),
        "wq": nc.dram_tensor("wq", [C, C], BF16, kind="ExternalInput").ap(),
        "wk": nc.dram_tensor("wk", [C, C], BF16, kind="ExternalInput").ap(),
        "wv": nc.dram_tensor("wv", [C, C], BF16, kind="ExternalInput").ap(),
        "pw": nc.dram_tensor("pw", [C, C], FP8, kind="ExternalInput").ap(),
        "bq": nc.dram_tensor("bq", [C], FP32, kind="ExternalInput").ap(),
        "bk": nc.dram_tensor("bk", [C], FP32, kind="ExternalInput").ap(),
        "pb": nc.dram_tensor("pb", [C], FP32, kind="ExternalInput").ap(),
        "gg": nc.dram_tensor("gg", [C], FP32, kind="ExternalInput").ap(),
        "gb": nc.dram_tensor("gb", [C], FP32, kind="ExternalInput").ap(),
        "amat": nc.dram_tensor("amat", [P, NH], FP32, kind="ExternalInput").ap(),
        "imat": nc.dram_tensor("imat", [NH, P], FP32, kind="ExternalInput").ap(),

        "ones8": nc.dram_tensor("ones8", [P, MT * NH * 128], FP8,
                                kind="ExternalInput").ap(),
        "out": nc.dram_tensor("out", [C, NT], FP32, kind="ExternalOutput").ap(),
    }
    with tile.TileContext(nc) as tc:
        _emit(tc, io)
    nc.compile()
    _CACHE["nc"] = nc
    return nc


def _host_prep(inputs):
    x = np.ascontiguousarray(np.asarray(inputs["x"], dtype=np.float32))
    qkv_w = np.asarray(inputs["qkv_w"], dtype=np.float32)
    qkv_b = np.asarray(inputs["qkv_b"], dtype=np.float32)
    proj_w = np.asarray(inputs["proj_w"], dtype=np.float32)
    proj_b = np.asarray(inputs["proj_b"], dtype=np.float32)
    gn_scale = np.asarray(inputs["gn_scale"], dtype=np.float32)
    gn_bias = np.asarray(inputs["gn_bias"], dtype=np.float32)

    s = np.float32(1.0 / np.sqrt(HD))
    bf = ml_dtypes.bfloat16
    f8 = ml_dtypes.float8_e4m3
    shared = {
        "wq": np.ascontiguousarray((qkv_w[0:C] * s).T).astype(bf),
        "wk": np.ascontiguousarray(qkv_w[C:2 * C].T).astype(bf),
        "wv": np.ascontiguousarray(qkv_w[2 * C:3 * C].T).astype(bf),
        "pw": np.ascontiguousarray(proj_w.T).astype(f8),
        "bq": (qkv_b[0:C] * s).astype(np.float32),
        "bk": qkv_b[C:2 * C].astype(np.float32),
        # v bias and proj bias folded: proj(o + b_v) = proj(o) + W_p b_v
        "pb": (proj_b + proj_w @ qkv_b[2 * C:3 * C]).astype(np.float32),
        "gg": gn_scale,
        "gb": gn_bias,
        "amat": (np.kron(np.eye(NH, dtype=np.float32),
                         np.ones((GSZ, 1), np.float32)) / GSZ),
        "imat": np.ascontiguousarray(np.kron(np.eye(NH, dtype=np.float32),
                                             np.ones((1, GSZ), np.float32))),
        "ones8": np.ones((P, MT * NH * 128), ml_dtypes.float8_e4m3),

    }
    B = x.shape[0]
    in_maps = []
    for b in range(B):
        m = dict(shared)
        m["x"] = np.ascontiguousarray(x[b].reshape(C, NT))
        in_maps.append(m)
    return in_maps


def run(inputs, trace=False):
    nc = _build()
    in_maps = _host_prep(inputs)
    res = run_bass_kernel_spmd(nc, in_maps, list(range(NCORES)), trace=trace)
    out = np.stack([res.results[i]["out"] for i in range(NCORES)], axis=0)
    return out.reshape(len(in_maps), C, 32, 32), res


def kernel(**inputs) -> np.ndarray:
    out, _ = run(inputs, trace=False)
    return out.astype(np.float32)
